# revision 2
# baseline (speedup 1.0000x reference)
"""Trainium2 Bass kernel for nn_CGAT — redesigned for low instruction count.

Structure (per core, SPMD over 8 cores):
  dense:  asd = x_slice @ Wa_sd (a_s/a_d logit parts per node), AllGather -> table1.
  L1:     313 global tiles of exactly 128 dsts (grid = dst//128), 40/core.
          Per tile: 3 chunked dma_gathers pull x rows per edge-slot (slots
          ordered chunk-major, idx-0 padded); sel built in one broadcast
          is_equal; selT/xT via PE transposes; l = xT.T@Wa_s + selT.T@a_d;
          pexp = exp(lrelu(l)); msg = x (x) pexp with a ones column; flip
          matmuls give agg[d, h*129] with denominators in col 128 of each
          head slab; normalize per dst, apply folded [M1|M1@Wa2], store
          bf16 h1 rows [h1|a_s2(f32)|1(f32)] + a2 rows; AllGather a2 -> table2
          (row = dst id).
  L2:     63 global tiles x 8 cores (partials, edges owned by src owner).
          One dma_gather per 3 tiles pulls 768B h1 rows; same sel/selT/flip
          matmul pipeline; ship [agg@M2 | denom] to rs_in via one strided
          store; ReduceScatter(add); finalize = divide + bias.
"""
import math
import numpy as np
import ml_dtypes

import concourse.bass as bass
from concourse.bass import DynSlice
from concourse import bacc
import concourse.tile as tile
from concourse import mybir
from concourse.bass_utils import run_bass_kernel_spmd

P = 128
C = 8
N0, N1, N2 = 80000, 40000, 8000
E1, E2 = 400000, 128000
F, HID, OUT, H = 128, 256, 128, 4
NEG = 0.2
BF16 = ml_dtypes.bfloat16

NSL = 5120                  # dense slice rows per core (= N1 / C rows of dsts)
DIT = NSL // P              # 40 dense iterations
T1G = math.ceil(N1 / P)     # 313 global L1 tiles
NT1 = math.ceil(T1G / C)    # 40 tiles per core
CHB = 26667                 # x-row chunk size (< 32768 for int16 gather idx)
T2G = math.ceil(N2 / P)     # 63 global L2 tiles
GB2 = 2                     # L2 tiles per gather group (num_idxs <= 1024)
NG2 = math.ceil(T2G / GB2)  # gather groups
T2GP = NG2 * GB2            # tiles incl. group padding
H1C = 384                   # h1 table row: 256 h1 bf16 | a_s2 f32 | 1.0 f32 | pad
RSP = 129                   # rs row: 128 out + denom


def _wrap16(vals, ncols):
    """int16 idx list -> [128, ncols] wrapped-16 replicated layout."""
    out = np.zeros((16, ncols), np.int16)
    j = np.arange(len(vals))
    out[j % 16, j // 16] = vals
    return np.tile(out, (8, 1))


def preprocess(inputs):
    x = np.ascontiguousarray(np.asarray(inputs["x"], np.float32))
    s1 = np.asarray(inputs["edge_src1"]).astype(np.int64)
    d1 = np.asarray(inputs["edge_dst1"]).astype(np.int64)
    s2 = np.asarray(inputs["edge_src2"]).astype(np.int64)
    d2 = np.asarray(inputs["edge_dst2"]).astype(np.int64)
    W1 = np.asarray(inputs["W1"], np.float32)
    att_s1 = np.asarray(inputs["att_src1"], np.float32)
    att_d1 = np.asarray(inputs["att_dst1"], np.float32)
    b1 = np.asarray(inputs["b1"], np.float32)
    W2 = np.asarray(inputs["W2"], np.float32)
    att_s2 = np.asarray(inputs["att_src2"], np.float32)
    att_d2 = np.asarray(inputs["att_dst2"], np.float32)
    b2 = np.asarray(inputs["b2"], np.float32)
    c1w = np.asarray(inputs["conv1_w"], np.float32)
    c1b = np.asarray(inputs["conv1_b"], np.float32)
    c2w = np.asarray(inputs["conv2_w"], np.float32)
    c2b = np.asarray(inputs["conv2_b"], np.float32)

    # ---- folded weights ----
    W1h = W1.reshape(F, H, F)
    Wa_sd1 = np.concatenate([np.einsum("fhc,hc->fh", W1h, att_s1),
                             np.einsum("fhc,hc->fh", W1h, att_d1)], axis=1)  # [128,8]
    c1wT = c1w.T
    M1 = np.stack([W1h[:, h, :] @ c1wT[h * F:(h + 1) * F] for h in range(H)])  # [4,128,256]
    cc1 = c1b + c1w @ b1
    Wa2 = np.stack([W2 @ att_s2[0], W2 @ att_d2[0]], axis=1)  # [256,2]
    M2 = W2 @ c2w.T                                            # [256,128]
    cc2 = c2b + c2w @ b2
    c1W2 = cc1 @ Wa2                                           # [2]
    # m1cat slab h: [M1_h | M1_h@Wa2]  -> [128, 4*258]
    m1cat = np.concatenate([np.concatenate([M1[h], M1[h] @ Wa2], axis=1)
                            for h in range(H)], axis=1).astype(BF16)
    m2cat = np.concatenate([M2[0:P, :], M2[P:2 * P, :]], axis=1).astype(BF16)

    # ---- L1 edge layout ----
    tiles = d1 // P                       # global tile of each edge
    chunk = s1 // CHB
    order = np.lexsort((d1, chunk, tiles))
    s1s, d1s, t1s, c1s = s1[order], d1[order], tiles[order], chunk[order]
    ntc = np.bincount(t1s * 3 + c1s, minlength=T1G * 3).reshape(T1G, 3)
    Bc = [max(1, int(math.ceil(ntc[:, c].max() / P))) for c in range(3)]
    BMAX = sum(Bc)
    Boff = [0, Bc[0], Bc[0] + Bc[1]]
    assert BMAX <= 16, f"BMAX={BMAX}"

    # rank of each edge within its (tile, chunk)
    key = t1s * 3 + c1s
    # edges are sorted by key; rank = position - first_position_of_key
    first = np.zeros(T1G * 3 + 1, np.int64)
    np.add.at(first, key + 1, 1)
    first = np.cumsum(first)
    rank = np.arange(len(s1s)) - first[key]

    core1 = t1s // NT1
    tl1 = t1s % NT1
    blk = np.array(Boff, np.int64)[c1s] + rank // P
    pos = rank % P
    j_in_chunk = rank                      # position within chunk segment

    m_idx1 = np.full((C, 16, NT1 * BMAX * 8), -1, np.int16)
    m_dstloc1 = np.full((C, P, NT1 * BMAX), 128.0, np.float32)
    m_cnt1 = np.zeros((C, 1, NT1 * 3), np.int32)
    colb = tl1 * (BMAX * 8) + np.array(Boff, np.int64)[c1s] * 8 + j_in_chunk // 16
    m_idx1[core1, j_in_chunk % 16, colb] = (s1s - c1s * CHB).astype(np.int16)
    m_dstloc1[core1, pos, tl1 * BMAX + blk] = (d1s - t1s * P).astype(np.float32)
    np.add.at(m_cnt1, (core1, 0, tl1 * 3 + c1s), 1)
    # empty (tile, chunk) segments: one idx-0 entry (gather of row 0, sel==0)
    for c in range(C):
        for seg in np.nonzero(m_cnt1[c, 0] == 0)[0]:
            tl0, ch0 = seg // 3, seg % 3
            m_idx1[c, 0, tl0 * (BMAX * 8) + Boff[ch0] * 8] = 0
            m_cnt1[c, 0, seg] = 1

    # ---- L2 edge layout ----
    own2 = s2 // NSL
    t2 = d2 // P
    B2 = 1
    pc = []
    for c in range(C):
        selm = own2 == c
        s2c, d2c, t2c = s2[selm], d2[selm], t2[selm]
        o = np.lexsort((d2c, t2c))
        s2c, d2c, t2c = s2c[o], d2c[o], t2c[o]
        cnt = np.bincount(t2c, minlength=T2G)
        B2 = max(B2, int(math.ceil(cnt.max() / P)))
        pc.append((s2c, d2c, t2c, cnt))
    m_idx2 = np.full((C, 16, T2GP * B2 * 8), -1, np.int16)
    m_cnt2 = np.zeros((C, 1, T2GP), np.int32)
    m_dstloc2 = np.full((C, P, T2GP * B2), 128.0, np.float32)
    for c in range(C):
        s2c, d2c, t2c, cnt = pc[c]
        first2 = np.zeros(T2G + 1, np.int64)
        first2[1:] = np.cumsum(cnt)
        rank2 = np.arange(len(s2c)) - first2[t2c]
        colb2 = t2c * (B2 * 8) + rank2 // 16
        m_idx2[c, rank2 % 16, colb2] = (s2c - c * NSL).astype(np.int16)
        m_dstloc2[c, rank2 % P, t2c * B2 + rank2 // P] = (d2c - t2c * P).astype(np.float32)
        m_cnt2[c, 0, 0:T2G] = cnt
    for c in range(C):
        for t0 in np.nonzero(m_cnt2[c, 0] == 0)[0]:
            m_idx2[c, 0, t0 * (B2 * 8)] = 0
            m_cnt2[c, 0, t0] = 1

    x_bf16 = x.astype(BF16)

    # ---- constants ----
    irow = np.tile(np.arange(P, dtype=np.float32), (P, 1)).astype(BF16)
    identb = np.eye(P, dtype=np.float32).astype(BF16)
    identf = np.eye(P, dtype=np.float32)
    c1bt = np.tile(cc1[None, :], (P, 1)).astype(np.float32)
    c1w2t = np.tile(c1W2[None, :], (P, 1)).astype(np.float32)
    cc2t = np.tile(cc2[None, :], (P, 1)).astype(np.float32)

    in_maps = []
    for c in range(C):
        xs = np.zeros((NSL, F), np.float32)
        lo = c * NSL
        hi = min(N0, lo + NSL)
        xs[0:hi - lo] = x[lo:hi]
        in_maps.append(dict(
            x_bf16=x_bf16, x_slice=xs,
            m_idx1=np.tile(m_idx1[c], (8, 1)),
            m_dstloc1=m_dstloc1[c].astype(BF16),
            m_idx2=np.tile(m_idx2[c], (8, 1)),
            m_dstloc2=m_dstloc2[c].astype(BF16),
            m_cnt1=m_cnt1[c], m_cnt2=m_cnt2[c],
            wasd=Wa_sd1, wasd_sb=Wa_sd1[:, 0:H].astype(BF16),
            m1cat=m1cat, m2cat=m2cat,
            c1bt=c1bt, c1w2t=c1w2t, cc2t=cc2t,
            irow=irow, identb=identb, identf=identf,
        ))
    plan = dict(B0=Bc[0], B1=Bc[1], B2=Bc[2], BL2=B2)
    return in_maps, plan


def build_kernel(B0, B1, B2, BL2):
    nc = bacc.Bacc("TRN2", target_bir_lowering=False, debug=False, num_devices=C)
    dt = mybir.dt
    AF = mybir.ActivationFunctionType
    AL = mybir.AluOpType
    BMAX = B0 + B1 + B2
    Boff = [0, B0, B0 + B1]
    Bcs = [B0, B1, B2]
    SL1 = BMAX * P           # L1 slots per tile
    SL2 = BL2 * P            # L2 slots per tile
    NI2G = GB2 * SL2         # idxs per L2 gather group

    x_bf16 = nc.declare_dram_parameter("x_bf16", [N0, F], dt.bfloat16, isOutput=False)
    x_slice = nc.declare_dram_parameter("x_slice", [NSL, F], dt.float32, isOutput=False)
    m_idx1 = nc.declare_dram_parameter("m_idx1", [P, NT1 * BMAX * 8], dt.int16, isOutput=False)
    m_dstloc1 = nc.declare_dram_parameter("m_dstloc1", [P, NT1 * BMAX], dt.bfloat16, isOutput=False)
    m_idx2 = nc.declare_dram_parameter("m_idx2", [P, T2GP * BL2 * 8], dt.int16, isOutput=False)
    m_cnt1 = nc.declare_dram_parameter("m_cnt1", [1, NT1 * 3], dt.int32, isOutput=False)
    m_cnt2 = nc.declare_dram_parameter("m_cnt2", [1, T2GP], dt.int32, isOutput=False)
    m_dstloc2 = nc.declare_dram_parameter("m_dstloc2", [P, T2GP * BL2], dt.bfloat16, isOutput=False)
    wasd = nc.declare_dram_parameter("wasd", [F, 2 * H], dt.float32, isOutput=False)
    wasd_sb = nc.declare_dram_parameter("wasd_sb", [F, H], dt.bfloat16, isOutput=False)
    m1cat = nc.declare_dram_parameter("m1cat", [F, H * (HID + 2)], dt.bfloat16, isOutput=False)
    m2cat = nc.declare_dram_parameter("m2cat", [P, 2 * OUT], dt.bfloat16, isOutput=False)
    c1bt = nc.declare_dram_parameter("c1bt", [P, HID], dt.float32, isOutput=False)
    c1w2t = nc.declare_dram_parameter("c1w2t", [P, 2], dt.float32, isOutput=False)
    cc2t = nc.declare_dram_parameter("cc2t", [P, OUT], dt.float32, isOutput=False)
    irow = nc.declare_dram_parameter("irow", [P, P], dt.bfloat16, isOutput=False)
    identb = nc.declare_dram_parameter("identb", [P, P], dt.bfloat16, isOutput=False)
    identf = nc.declare_dram_parameter("identf", [P, P], dt.float32, isOutput=False)
    out_ext = nc.declare_dram_parameter("out", [N2 // C, OUT], dt.float32, isOutput=True)

    asd_slice = nc.dram_tensor("asd_slice", [NSL, 2 * H], dt.float32)
    table1 = nc.dram_tensor("table1", [C * NSL, 2 * H], dt.float32, addr_space="Shared")
    h1tab = nc.dram_tensor("h1tab", [NSL, H1C], dt.bfloat16)
    a2_slice = nc.dram_tensor("a2_slice", [NSL, 2], dt.float32)
    table2 = nc.dram_tensor("table2", [C * NSL, 2], dt.float32, addr_space="Shared")
    rs_in = nc.dram_tensor("rs_in", [T2G * P, RSP], dt.float32)
    rs_out = nc.dram_tensor("rs_out", [1024, RSP], dt.float32)

    with nc.allow_low_precision(reason="bf16 softmax weights by design"), \
         tile.TileContext(nc) as tc:
        with tc.tile_pool(name="const", bufs=1) as cpool, \
             tc.tile_pool(name="meta", bufs=1) as mpool:
            def load(pool, src_ap, shape, dtype, tag):
                t = pool.tile(shape, dtype, tag=tag)
                nc.sync.dma_start(t[:], src_ap)
                return t

            t_irow = load(cpool, irow[:], [P, P], dt.bfloat16, tag="t_irow")
            t_identb = load(cpool, identb[:], [P, P], dt.bfloat16, tag="t_identb")
            t_identf = load(cpool, identf[:], [P, P], dt.float32, tag="t_identf")
            t_wasd = load(cpool, wasd[:], [F, 2 * H], dt.float32, tag="t_wasd")
            t_wasdsb = load(cpool, wasd_sb[:], [F, H], dt.bfloat16, tag="t_wasdsb")
            t_m1cat = load(cpool, m1cat[:], [F, H * (HID + 2)], dt.bfloat16, tag="t_m1cat")
            t_m2cat = load(cpool, m2cat[:], [P, 2 * OUT], dt.bfloat16, tag="t_m2cat")
            t_c1bt = load(cpool, c1bt[:], [P, HID], dt.float32, tag="t_c1bt")
            t_c1w2t = load(cpool, c1w2t[:], [P, 2], dt.float32, tag="t_c1w2t")
            t_cc2t = load(cpool, cc2t[:], [P, OUT], dt.float32, tag="t_cc2t")
            t_idx1 = load(mpool, m_idx1[:], [P, NT1 * BMAX * 8], dt.int16, tag="t_idx1")
            t_dl1 = load(mpool, m_dstloc1[:], [P, NT1 * BMAX], dt.bfloat16, tag="t_dl1")
            t_idx2 = load(mpool, m_idx2[:], [P, T2GP * BL2 * 8], dt.int16, tag="t_idx2")
            t_dl2 = load(mpool, m_dstloc2[:], [P, T2GP * BL2], dt.bfloat16, tag="t_dl2")
            t_cnt1 = load(mpool, m_cnt1[:], [1, NT1 * 3], dt.int32, tag="t_cnt1")
            t_cnt2 = load(mpool, m_cnt2[:], [1, T2GP], dt.int32, tag="t_cnt2")

            # =================== dense: asd table ===================
            sc = nc.enter_named_scope("dense", False)
            with tc.tile_pool(name="dwork", bufs=2) as dwp, \
                 tc.tile_pool(name="dacc", bufs=1) as dap, \
                 tc.tile_pool(name="dps", bufs=2, space="PSUM") as dps:
                asd_acc = dap.tile([P, DIT * 2 * H], dt.float32, tag="asd_acc")
                for jj in range(DIT // 4):
                    xd = dwp.tile([P, 4 * F], dt.float32, tag="xd")
                    nc.sync.dma_start(
                        xd[:].rearrange("p (b f) -> p b f", f=F),
                        x_slice[:].rearrange("(b q) f -> q b f", q=P)[:, jj * 4:(jj + 1) * 4, :])
                    for b in range(4):
                        j = jj * 4 + b
                        pxT = dps.tile([P, P], dt.float32, space="PSUM", tag="pxT")
                        nc.tensor.transpose(out=pxT[:], in_=xd[:, b * F:(b + 1) * F],
                                            identity=t_identf[:])
                        xTs = dwp.tile([P, P], dt.float32, tag="xTs")
                        nc.vector.tensor_copy(out=xTs[:], in_=pxT[:])
                        pa = dps.tile([P, 2 * H], dt.float32, space="PSUM", tag="pa")
                        nc.tensor.matmul(pa[:], lhsT=xTs[:], rhs=t_wasd[:],
                                         start=True, stop=True)
                        nc.vector.tensor_copy(out=asd_acc[:, j * 8:(j + 1) * 8], in_=pa[:])
                nc.sync.dma_start(
                    asd_slice[:].rearrange("(b q) k -> q b k", q=P),
                    asd_acc[:].rearrange("p (b k) -> p b k", k=8))
            nc.leave_named_scope("dense", sc[0], False)

            sc = nc.enter_named_scope("ag1", False)
            nc.gpsimd.collective_compute(
                "AllGather", AL.bypass, replica_groups=[list(range(C))],
                ins=[asd_slice[:]], outs=[table1[:]])
            nc.leave_named_scope("ag1", sc[0], False)

            # =================== layer-1 tiles ===================
            sc = nc.enter_named_scope("l1", False)
            pid = nc.sync.partition_id()
            pidbase = pid * NSL
            with tc.tile_pool(name="l1w", bufs=3) as wp, \
                 tc.tile_pool(name="l1s", bufs=3) as sp, \
                 tc.tile_pool(name="l1pt", bufs=2, space="PSUM") as pst, \
                 tc.tile_pool(name="l1pp", bufs=1, space="PSUM") as psp:
                for _w in range(3):
                    xab_w = wp.tile([P, SL1], dt.bfloat16, tag="xab", name="xab_w")
                    nc.vector.memset(xab_w[:], 0.0)
                for tl in range(NT1):
                    # a_d rows for this tile's 128 dsts (row == dst id)
                    adt = sp.tile([P, H], dt.float32, tag="adt")
                    nc.sync.dma_start(adt[:], table1[DynSlice(pidbase + tl * P, P), H:2 * H])
                    adtb = sp.tile([P, H], dt.bfloat16, tag="adtb")
                    nc.scalar.activation(adtb[:], adt[:], AF.Copy)

                    # gather x rows (3 chunks) straight into bf16
                    xab = wp.tile([P, SL1], dt.bfloat16, tag="xab")
                    for ch in range(3):
                        nb = Bcs[ch]
                        i0 = tl * (BMAX * 8) + Boff[ch] * 8
                        rl = CHB * ch
                        rh = min(N0, CHB * (ch + 1))
                        rcg = nc.gpsimd.alloc_register(f"cnt1_{tl}_{ch}")
                        nc.gpsimd.reg_load(rcg, t_cnt1[0:1, tl * 3 + ch:tl * 3 + ch + 1])
                        ncnt = nc.gpsimd.snap(rcg, donate=True, min_val=0, max_val=nb * P)
                        nc.gpsimd.dma_gather(
                            out_ap=xab[:, Boff[ch] * F:(Boff[ch] + nb) * F]
                                .rearrange("p (b f) -> p b f", f=F),
                            in_ap=x_bf16[rl:rh, :],
                            idxs_ap=t_idx1[:, i0:i0 + nb * 8],
                            num_idxs=nb * P, num_idxs_reg=ncnt, elem_size=F)

                    # sel [e, (b, d)] one op
                    sel = wp.tile([P, SL1], dt.bfloat16, tag="sel")
                    nc.vector.tensor_tensor(
                        out=sel[:].rearrange("p (b d) -> p b d", d=P),
                        in0=bass.AP(t_irow.tensor, t_irow[:].offset,
                                    [t_irow[:].ap[0], [0, BMAX], [1, P]]),
                        in1=bass.AP(t_dl1.tensor, t_dl1[:].offset + tl * BMAX,
                                    [t_dl1[:].ap[0], [1, BMAX], [0, P]]),
                        op=AL.is_equal)

                    # selT via PE transposes
                    ptr = pst.tile([P, SL1], dt.bfloat16, space="PSUM", tag="ptr")
                    for b in range(BMAX):
                        nc.tensor.transpose(out=ptr[:, b * P:(b + 1) * P],
                                            in_=sel[:, b * P:(b + 1) * P],
                                            identity=t_identb[:])
                    selT = wp.tile([P, SL1], dt.bfloat16, tag="selT")
                    nc.scalar.activation(selT[:], ptr[:], AF.Copy)
                    # xT via PE transposes (same psum ring)
                    ptr2 = pst.tile([P, SL1], dt.bfloat16, space="PSUM", tag="ptr")
                    for b in range(BMAX):
                        nc.tensor.transpose(
                            out=ptr2[:, b * P:(b + 1) * P],
                            in_=xab[:, b * F:(b + 1) * F],
                            identity=t_identb[:])
                    xts = wp.tile([P, SL1], dt.bfloat16, tag="xts")
                    nc.scalar.activation(xts[:], ptr2[:], AF.Copy)

                    # logits l = a_s + a_d  [e, (b, h)]
                    pl = psp.tile([P, BMAX * H], dt.float32, space="PSUM", tag="pl")
                    for b in range(BMAX):
                        nc.tensor.matmul(pl[:, b * H:(b + 1) * H],
                                         lhsT=xts[:, b * P:(b + 1) * P],
                                         rhs=t_wasdsb[:], start=True, stop=False)
                        nc.tensor.matmul(pl[:, b * H:(b + 1) * H],
                                         lhsT=selT[:, b * P:(b + 1) * P],
                                         rhs=adtb[:], start=False, stop=True)
                    lsb = sp.tile([P, BMAX * H], dt.float32, tag="lsb")
                    nc.vector.tensor_copy(out=lsb[:], in_=pl[:])
                    lr = sp.tile([P, BMAX * H], dt.float32, tag="lr")
                    nc.vector.scalar_tensor_tensor(out=lr[:], in0=lsb[:], scalar=NEG,
                                                   in1=lsb[:], op0=AL.mult, op1=AL.max)
                    pexp = sp.tile([P, BMAX * H], dt.float32, tag="pexp")
                    nc.scalar.activation(pexp[:], lr[:], AF.Exp)
                    pexpb = sp.tile([P, BMAX * H], dt.bfloat16, tag="pexpb")
                    nc.vector.tensor_copy(out=pexpb[:], in_=pexp[:])

                    # msg = x (x) pexp  [e, (b, h, 128)]
                    msgb = wp.tile([P, BMAX * H * F], dt.bfloat16, tag="msgb")
                    ng = math.ceil(BMAX / 4)
                    for g in range(ng):
                        gb = min(4, BMAX - g * 4)
                        nc.vector.tensor_tensor(
                            out=msgb[:, g * 4 * H * F:(g * 4 + gb) * H * F]
                                .rearrange("p (b h f) -> p b h f", h=H, f=F),
                            in0=bass.AP(xab.tensor, xab[:].offset + g * 4 * F,
                                        [xab[:].ap[0], [F, gb], [0, H], [1, F]]),
                            in1=bass.AP(pexpb.tensor, pexpb[:].offset + g * 4 * H,
                                        [pexpb[:].ap[0], [H, gb], [1, H], [0, F]]),
                            op=AL.mult)

                    # flip matmuls: pagg[d, (h, 128)] single 512-wide psum;
                    # denominators via separate tiny matmuls into pdp.
                    pagg = psp.tile([P, H * F], dt.float32, space="PSUM", tag="pagg")
                    pdp = psp.tile([P, H], dt.float32, space="PSUM", tag="pdp")
                    for b in range(BMAX):
                        nc.tensor.matmul(pagg[:], lhsT=sel[:, b * P:(b + 1) * P],
                                         rhs=msgb[:, b * H * F:(b + 1) * H * F],
                                         start=(b == 0), stop=(b == BMAX - 1))
                        nc.tensor.matmul(pdp[:], lhsT=sel[:, b * P:(b + 1) * P],
                                         rhs=pexpb[:, b * H:(b + 1) * H],
                                         start=(b == 0), stop=(b == BMAX - 1))
                    dsb = sp.tile([P, H], dt.float32, tag="dsb")
                    nc.scalar.activation(dsb[:], pdp[:], AF.Copy, bias=1e-16)
                    rec = sp.tile([P, H], dt.float32, tag="rec")
                    nc.vector.reciprocal(out=rec[:], in_=dsb[:])
                    # normalize per dst -> bf16 [d, (h, 128)]
                    agg2s = sp.tile([P, H * F], dt.bfloat16, tag="agg2s")
                    nc.vector.tensor_tensor(
                        out=agg2s[:].rearrange("p (h f) -> p h f", f=F),
                        in0=pagg[:].rearrange("p (h f) -> p h f", f=F),
                        in1=bass.AP(rec.tensor, rec[:].offset,
                                    [rec[:].ap[0], [1, H], [0, F]]),
                        op=AL.mult)
                    # transpose agg slabs -> [f, d]
                    ptr3 = pst.tile([P, SL1], dt.bfloat16, space="PSUM", tag="ptr")
                    for h in range(H):
                        nc.tensor.transpose(out=ptr3[:, h * P:(h + 1) * P],
                                            in_=agg2s[:, h * F:(h + 1) * F],
                                            identity=t_identb[:])
                    aggT = sp.tile([P, H * P], dt.bfloat16, tag="aggT")
                    nc.scalar.activation(aggT[:], ptr3[:, 0:H * P], AF.Copy)
                    # h1/a2 = sum_h aggT_h.T @ [M1_h | M1_h Wa2]
                    ph1 = psp.tile([P, HID + 2], dt.float32, space="PSUM", tag="ph1")
                    for h in range(H):
                        nc.tensor.matmul(ph1[:], lhsT=aggT[:, h * P:(h + 1) * P],
                                         rhs=t_m1cat[:, h * (HID + 2):(h + 1) * (HID + 2)],
                                         start=(h == 0), stop=(h == H - 1))
                    h1b = wp.tile([P, H1C], dt.bfloat16, tag="h1b")
                    nc.vector.tensor_tensor(out=h1b[:, 0:HID], in0=ph1[:, 0:HID],
                                            in1=t_c1bt[:], op=AL.add)
                    a2sb = sp.tile([P, 2], dt.float32, tag="a2sb")
                    nc.vector.tensor_tensor(out=a2sb[:], in0=ph1[:, HID:HID + 2],
                                            in1=t_c1w2t[:], op=AL.add)
                    h1f32 = h1b[:].bitcast(dt.float32)   # [P, 192] f32 view
                    nc.vector.tensor_copy(
                        out=bass.AP(h1f32.tensor, h1f32.offset + 128, [h1f32.ap[0], [1, 1]]),
                        in_=a2sb[:, 0:1])
                    nc.vector.memset(
                        bass.AP(h1f32.tensor, h1f32.offset + 129, [h1f32.ap[0], [1, 1]]), 1.0)
                    nc.sync.dma_start(h1tab[tl * P:(tl + 1) * P, :], h1b[:])
                    nc.sync.dma_start(a2_slice[tl * P:(tl + 1) * P, :], a2sb[:])
            nc.leave_named_scope("l1", sc[0], False)

            sc = nc.enter_named_scope("ag2", False)
            nc.gpsimd.collective_compute(
                "AllGather", AL.bypass, replica_groups=[list(range(C))],
                ins=[a2_slice[:]], outs=[table2[:]])
            nc.leave_named_scope("ag2", sc[0], False)

            # =================== layer-2 tiles ===================
            sc = nc.enter_named_scope("l2", False)
            with tc.tile_pool(name="l2w", bufs=3) as wp2, \
                 tc.tile_pool(name="l2s", bufs=3) as sp2, \
                 tc.tile_pool(name="l2acc", bufs=1) as ap2, \
                 tc.tile_pool(name="l2pt", bufs=2, space="PSUM") as pst2, \
                 tc.tile_pool(name="l2pp", bufs=1, space="PSUM") as psp2:
                # a_d2 per dst: table2 row = dst id, col 1
                ad2 = ap2.tile([P, T2G], dt.float32, tag="ad2")
                nc.sync.dma_start(
                    ad2[:],
                    bass.AP(table2, 1, [[2, P], [2 * P, T2G]]))
                ad2b = ap2.tile([P, T2G], dt.bfloat16, tag="ad2b")
                nc.vector.tensor_copy(out=ad2b[:], in_=ad2[:])
                o2all = ap2.tile([P, T2G * RSP], dt.float32, tag="o2all")

                for _w in range(3):
                    xg_w = wp2.tile([P, GB2 * BL2 * H1C], dt.bfloat16, tag="xg", name="xg_w")
                    nc.vector.memset(xg_w[:], 0.0)
                for g in range(NG2):
                    xg = wp2.tile([P, GB2 * BL2 * H1C], dt.bfloat16, tag="xg")
                    for ti2 in range(GB2):
                        t2i = g * GB2 + ti2
                        i0 = t2i * BL2 * 8
                        rcg2 = nc.gpsimd.alloc_register(f"cnt2_{t2i}")
                        nc.gpsimd.reg_load(rcg2, t_cnt2[0:1, t2i:t2i + 1])
                        ncnt2 = nc.gpsimd.snap(rcg2, donate=True, min_val=0, max_val=SL2)
                        nc.gpsimd.dma_gather(
                            out_ap=xg[:, ti2 * BL2 * H1C:(ti2 + 1) * BL2 * H1C]
                                .rearrange("p (b f) -> p b f", f=H1C),
                            in_ap=h1tab[:],
                            idxs_ap=t_idx2[:, i0:i0 + BL2 * 8],
                            num_idxs=SL2, num_idxs_reg=ncnt2, elem_size=H1C)
                    xg32 = xg[:].bitcast(dt.float32)   # [P, GB2*BL2*192]
                    for ti in range(GB2):
                        t = g * GB2 + ti
                        if t >= T2G:
                            break
                        sel2 = wp2.tile([P, SL2], dt.bfloat16, tag="sel2")
                        nc.vector.tensor_tensor(
                            out=sel2[:].rearrange("p (b d) -> p b d", d=P),
                            in0=bass.AP(t_irow.tensor, t_irow[:].offset,
                                        [t_irow[:].ap[0], [0, BL2], [1, P]]),
                            in1=bass.AP(t_dl2.tensor, t_dl2[:].offset + t * BL2,
                                        [t_dl2[:].ap[0], [1, BL2], [0, P]]),
                            op=AL.is_equal)
                        ptrA = pst2.tile([P, SL2], dt.bfloat16, space="PSUM", tag="ptrA")
                        for b in range(BL2):
                            nc.tensor.transpose(out=ptrA[:, b * P:(b + 1) * P],
                                                in_=sel2[:, b * P:(b + 1) * P],
                                                identity=t_identb[:])
                        selT2 = wp2.tile([P, SL2], dt.bfloat16, tag="selT2")
                        nc.scalar.activation(selT2[:], ptrA[:], AF.Copy)
                        pl2 = psp2.tile([P, BL2], dt.float32, space="PSUM", tag="pl2")
                        for b in range(BL2):
                            nc.tensor.matmul(pl2[:, b:b + 1],
                                             lhsT=selT2[:, b * P:(b + 1) * P],
                                             rhs=ad2b[:, t:t + 1], start=True, stop=True)
                        l2sb = sp2.tile([P, BL2], dt.float32, tag="l2sb")
                        nc.vector.tensor_tensor(
                            out=l2sb[:], in0=pl2[:],
                            in1=bass.AP(xg32.tensor,
                                        xg32.offset + ti * BL2 * (H1C // 2) + F,
                                        [xg32.ap[0], [H1C // 2, BL2]]),
                            op=AL.add)
                        lr2 = sp2.tile([P, BL2], dt.float32, tag="lr2")
                        nc.vector.scalar_tensor_tensor(out=lr2[:], in0=l2sb[:], scalar=NEG,
                                                       in1=l2sb[:], op0=AL.mult, op1=AL.max)
                        pexp2 = sp2.tile([P, BL2], dt.float32, tag="pexp2")
                        nc.scalar.activation(pexp2[:], lr2[:], AF.Exp)
                        pexp2b = sp2.tile([P, BL2], dt.bfloat16, tag="pexp2b")
                        nc.vector.tensor_copy(out=pexp2b[:], in_=pexp2[:])
                        # msg2 = h1row (x) pexp2; cols 0:260 (256 h1, 2 junk, 0, one)
                        W2C = HID + 4
                        msg2 = wp2.tile([P, BL2 * W2C], dt.bfloat16, tag="msg2")
                        nc.vector.tensor_tensor(
                            out=msg2[:].rearrange("p (b f) -> p b f", f=W2C),
                            in0=bass.AP(xg.tensor, xg[:].offset + ti * BL2 * H1C,
                                        [xg[:].ap[0], [H1C, BL2], [1, W2C]]),
                            in1=bass.AP(pexp2b.tensor, pexp2b[:].offset,
                                        [pexp2b[:].ap[0], [1, BL2], [0, W2C]]),
                            op=AL.mult)
                        pagg2 = psp2.tile([P, W2C], dt.float32, space="PSUM", tag="pagg2")
                        for b in range(BL2):
                            nc.tensor.matmul(pagg2[:], lhsT=sel2[:, b * P:(b + 1) * P],
                                             rhs=msg2[:, b * W2C:(b + 1) * W2C],
                                             start=(b == 0), stop=(b == BL2 - 1))
                        aggsb = sp2.tile([P, HID], dt.bfloat16, tag="aggsb")
                        nc.scalar.activation(aggsb[:], pagg2[:, 0:HID], AF.Copy)
                        ptrB = pst2.tile([P, 2 * P], dt.bfloat16, space="PSUM", tag="ptrB")
                        for k in range(2):
                            nc.tensor.transpose(out=ptrB[:, k * P:(k + 1) * P],
                                                in_=aggsb[:, k * P:(k + 1) * P],
                                                identity=t_identb[:])
                        aggT2 = sp2.tile([P, 2 * P], dt.bfloat16, tag="aggT2")
                        nc.scalar.activation(aggT2[:], ptrB[:], AF.Copy)
                        po2 = psp2.tile([P, OUT], dt.float32, space="PSUM", tag="po2")
                        for k in range(2):
                            nc.tensor.matmul(po2[:], lhsT=aggT2[:, k * P:(k + 1) * P],
                                             rhs=t_m2cat[:, k * OUT:(k + 1) * OUT],
                                             start=(k == 0), stop=(k == 1))
                        nc.vector.tensor_copy(out=o2all[:, t * RSP:t * RSP + OUT], in_=po2[:])
                        nc.vector.tensor_copy(
                            out=o2all[:, t * RSP + OUT:t * RSP + OUT + 1],
                            in_=pagg2[:, HID + 3:HID + 4])
                # one big strided store: rs_in[t*128+p, :] = o2all[p, t, :]
                nc.sync.dma_start(
                    rs_in[:].rearrange("(b q) k -> q b k", q=P),
                    o2all[:].rearrange("p (b k) -> p b k", k=RSP))
            nc.leave_named_scope("l2", sc[0], False)

            sc = nc.enter_named_scope("rs", False)
            nc.gpsimd.collective_compute(
                "ReduceScatter", AL.add, replica_groups=[list(range(C))],
                ins=[rs_in[0:N2, :]], outs=[rs_out[0:N2 // C, :]])
            nc.leave_named_scope("rs", sc[0], False)

            # =================== finalize ===================
            sc = nc.enter_named_scope("fin", False)
            NOUT = N2 // C
            with tc.tile_pool(name="fwork", bufs=1) as fwp:
                fin = fwp.tile([P, 8 * RSP], dt.float32, tag="fin")
                nc.sync.dma_start(
                    fin[:].rearrange("p (b k) -> p b k", k=RSP),
                    rs_out[:].rearrange("(b q) k -> q b k", q=P))
                dsf = fwp.tile([P, 8], dt.float32, tag="dsf")
                nc.scalar.activation(
                    dsf[:],
                    bass.AP(fin.tensor, fin[:].offset + OUT, [fin[:].ap[0], [RSP, 8]]),
                    AF.Copy, bias=1e-16)
                recf = fwp.tile([P, 8], dt.float32, tag="recf")
                nc.vector.reciprocal(out=recf[:], in_=dsf[:])
                osb = fwp.tile([P, 8 * OUT], dt.float32, tag="osb")
                nc.vector.tensor_tensor(
                    out=osb[:].rearrange("p (b k) -> p b k", k=OUT),
                    in0=bass.AP(fin.tensor, fin[:].offset, [fin[:].ap[0], [RSP, 8], [1, OUT]]),
                    in1=bass.AP(recf.tensor, recf[:].offset, [recf[:].ap[0], [1, 8], [0, OUT]]),
                    op=AL.mult)
                nc.vector.tensor_tensor(
                    out=osb[:].rearrange("p (b k) -> p b k", k=OUT),
                    in0=osb[:].rearrange("p (b k) -> p b k", k=OUT),
                    in1=bass.AP(t_cc2t.tensor, t_cc2t[:].offset,
                                [t_cc2t[:].ap[0], [0, 8], [1, OUT]]),
                    op=AL.add)
                nc.sync.dma_start(
                    out_ext[0:7 * P, :].rearrange("(b q) k -> q b k", q=P),
                    osb[:, 0:7 * OUT].rearrange("p (b k) -> p b k", k=OUT))
                nc.sync.dma_start(out_ext[7 * P:NOUT, :],
                                  osb[0:NOUT - 7 * P, 7 * OUT:8 * OUT])
            nc.leave_named_scope("fin", sc[0], False)

    nc.compile()
    return nc


_CACHE = {}


def kernel(**inputs) -> np.ndarray:
    in_maps, plan = preprocess(inputs)
    key = (plan["B0"], plan["B1"], plan["B2"], plan["BL2"])
    if key not in _CACHE:
        _CACHE[key] = build_kernel(*key)
    nc = _CACHE[key]
    res = run_bass_kernel_spmd(nc, in_maps, list(range(C))).results
    out = np.concatenate([res[c]["out"] for c in range(C)], axis=0)
    return out.astype(np.float32)
